# revision 53
# baseline (speedup 1.0000x reference)
"""Trainium2 8-core kernel for nn_Attention_27530740367526.

Multi-head causal attention (B=2, S=2048, D=2048, H=16, HD=128, fp32) with
RoPE, sharded batch x head-group across 8 NeuronCores: core c handles batch
c//4 and heads [4*(c%4), 4*(c%4)+4).  Each core computes q/k/v projections
(+RoPE), attention for its heads, and the slice of the wo projection those
heads feed — per-head-pair partial [S, D] outputs.  The host sums the 8
partials per batch (row-parallel wo "all-reduce" as a host-side unshard).

On-device everything lives in "transposed land": qT/kT are [head_dim, seq]
with head-dim on partitions, so scores come out transposed ([k, q]), and
PV / wo consume natural layouts with zero on-device transposes.  RoPE's
rotate-half is a 128x128 permutation matmul on the PE.

Schedule notes: all matmul operands and the y partials are bfloat16 (half
the DMA bytes, fast weight loads; attention math accumulates in fp32
PSUM).  The two heads of a pair share two-bank PSUM tiles ([P, 2, 512])
so exp / denominator-accumulate / reciprocal run once per k-block instead
of twice (halves the fixed per-instruction overhead on ACT/DVE).  The
softmax denominator is accumulated across k-blocks as a uniform-bf16
chain on DVE (mixed-width DVE tensor ops run ~2.5x slower) and broadcast
with a single ones-matmul per chunk whose reciprocal/normalize is
deferred into the next chunk's projection matmuls.  Fully-masked columns
of diagonal score blocks are never computed.  The next head-pair's
weights prefetch during the current pair's attention (double-buffered) so
the PE never starves at the pair boundary (keeps the HAM clock-gate at
8/8).  wo-projection filler blocks drain 1-2 per attention k-block where
the PE would otherwise wait on exp; leftovers carry across the pair
boundary.  Startup DMAs issue fine-grained and need-ordered across the
SP/ACT/Pool DGE queues (the ACT queue carries no late DMAs — a DMA issue
can block many us on ring credits, stalling compute queued behind it).
"""

import sys

if "/opt/trn_rl_repo" not in sys.path:
    sys.path.insert(0, "/opt/trn_rl_repo")

from collections import deque

import numpy as np
import ml_dtypes

import concourse.bacc as bacc
import concourse.mybir as mybir
import concourse.tile as tile
from concourse.bass_utils import run_bass_kernel_spmd

F32 = mybir.dt.float32
F32R = mybir.dt.float32r
BF16 = mybir.dt.bfloat16
AF = mybir.ActivationFunctionType

N_HEADS = 16
N_CORES = 8
B, S, D = 2, 2048, 2048
HD = D // N_HEADS
H_LOC = N_HEADS // (N_CORES // B)  # 4 heads per core
HW = H_LOC * HD                    # 512 q/k/v columns per core
SC = 512                           # seq chunk (matmul moving free dim)
P = 128


def _round_f32r(x: np.ndarray) -> np.ndarray:
    """Host-side fp32 -> float32r rounding (RNE to 11 explicit mantissa
    bits); bit-exact with the device DVE rounding."""
    xi = np.ascontiguousarray(x, dtype=np.float32).view(np.uint32)
    nbits = 12
    lo = np.uint32((1 << nbits) - 1)
    half = np.uint32(1 << (nbits - 1))
    rem = xi & lo
    up = (rem > half) | ((rem == half) & (((xi >> nbits) & 1) == 1))
    r = (xi & ~lo) + np.where(up, np.uint32(1 << nbits), np.uint32(0))
    return r.view(np.float32)


def _build_core_kernel():
    KO = D // P            # 16 contraction subtiles for projections
    NQC = S // SC          # 4 q-chunks
    NSUB = SC // P         # 4 128-blocks per chunk
    NST = S // P           # 16 s-tiles
    NHB = H_LOC // 2       # head pairs
    inv_sqrt_hd = 1.0 / float(np.sqrt(HD))

    nc = bacc.Bacc(None, target_bir_lowering=False)

    xT = nc.dram_tensor("xT", [D, S], BF16, kind="ExternalInput")
    wqkvT = nc.dram_tensor(
        "wqkvT", [NHB, D, 6 * HD], BF16, kind="ExternalInput"
    )
    woT = nc.dram_tensor("woT", [HW, D], BF16, kind="ExternalInput")
    cosT = nc.dram_tensor("cosT", [HD, S], F32, kind="ExternalInput")
    sinT = nc.dram_tensor("sinT", [HD, S], F32, kind="ExternalInput")
    PT = nc.dram_tensor("PT", [HD, HD], BF16, kind="ExternalInput")
    ones = nc.dram_tensor("ones", [P, P], BF16, kind="ExternalInput")
    # triangular additive mask for the 128x128 diagonal score blocks,
    # [k, q] orientation, pre-scaled by sqrt(HD)
    maskT = nc.dram_tensor("maskT", [P, P], F32, kind="ExternalInput")
    y = nc.dram_tensor("y", [NHB, S, D], BF16, kind="ExternalOutput")

    xT_r = xT.rearrange("(ko ki) s -> ki ko s", ki=P)
    wqkvT_r = wqkvT.rearrange("hb (ko ki) c -> hb ki ko c", ki=P)
    woT_r = woT.rearrange("(h ki) d -> ki h d", ki=P)

    with tile.TileContext(nc) as tc:
        with (
            tc.tile_pool(name="persist", bufs=1) as persist,
            tc.tile_pool(name="wpool", bufs=2) as wpool,
            tc.tile_pool(name="kvq", bufs=1) as kvq,
            tc.tile_pool(name="xa", bufs=2) as xa,
            tc.tile_pool(name="cs", bufs=2) as cspool,
            tc.tile_pool(name="scr", bufs=2) as scr,
            tc.tile_pool(name="exps", bufs=8) as expp,
            tc.tile_pool(name="qtp", bufs=2) as qtp,
            tc.tile_pool(name="outq", bufs=3) as outqp,
            tc.tile_pool(name="accs", bufs=2) as accp,
            tc.tile_pool(name="yo", bufs=6) as yop,
            tc.tile_pool(name="ps2", bufs=2, space="PSUM") as s2p,
            tc.tile_pool(name="ops", bufs=2, space="PSUM") as ops,
            tc.tile_pool(name="yps", bufs=2, space="PSUM") as yps,
        ):
            def load_cs(sc, eng=None):
                # SP queue by default: a DMA issue on the ACT queue can
                # block for many us on ring credits, stalling the compute
                # ops (plain copies / exps) queued behind it.  Two pieces
                # per tensor so a single transfer isn't 11us long.
                eng = eng or nc.sync
                cos_t = cspool.tile([P, SC], F32, tag="cos")
                sin_t = cspool.tile([P, SC], F32, tag="sin")
                for h in range(2):
                    l = slice(h * 256, (h + 1) * 256)
                    g = slice(sc * SC + h * 256, sc * SC + (h + 1) * 256)
                    eng.dma_start(cos_t[:, l], cosT[:, g])
                    eng.dma_start(sin_t[:, l], sinT[:, g])
                return cos_t, sin_t

            def load_x(sc):
                """x chunk DMAs (SP queue)."""
                ssl = slice(sc * SC, (sc + 1) * SC)
                xt = xa.tile([P, KO, SC], BF16, tag="xt")
                for ko in range(KO):
                    nc.sync.dma_start(xt[:, ko], xT_r[:, ko, ssl])
                return xt

            def load_w(hb):
                """wqkv DMAs for head pair hb (SP queue)."""
                w_sb = wpool.tile([P, KO, 6 * HD], BF16, tag="w")
                for ko in range(KO):
                    nc.sync.dma_start(w_sb[:, ko, :], wqkvT_r[hb, :, ko, :])
                return w_sb

            def load_wo(hb, eng):
                wo_sb = wpool.tile([P, 2, D], BF16, tag="wo")
                for hl in range(2):
                    for dc in range(4):
                        dsl = slice(dc * SC, (dc + 1) * SC)
                        eng.dma_start(
                            wo_sb[:, hl, dsl],
                            woT_r[:, hb * 2 + hl, dsl],
                        )
                return wo_sb

            # ---- startup: spread w/x issue across the SP / ACT / Pool
            # DGE queues, interleaved per-ko, so the first matmul group
            # can begin within a few us and streams at aggregate DMA bw.
            # Issue order is by need-time: x0 + w q/k wave first, then the
            # v columns, then x1 / cos/sin / wo.  The ACT queue carries
            # only the first w wave (its later compute must not sit
            # behind ring-credit-blocked DMA issues).
            w_cur = wpool.tile([P, KO, 6 * HD], BF16, tag="w", name="w_sb")
            xt0 = xa.tile([P, KO, SC], BF16, tag="xt", name="xt")
            # the first q-weight piece + x pieces lead their queues so the
            # first matmul group can start as early as possible
            nc.sync.dma_start(w_cur[:, 0, 0:HD], wqkvT_r[0, :, 0, 0:HD])
            for i in range(2):
                nc.sync.dma_start(
                    xt0[:, 0, i * P : (i + 1) * P],
                    xT_r[:, 0, i * P : (i + 1) * P],
                )
            for i in range(2, 4):
                nc.gpsimd.dma_start(
                    xt0[:, 0, i * P : (i + 1) * P],
                    xT_r[:, 0, i * P : (i + 1) * P],
                )
            nc.scalar.dma_start(
                w_cur[:, 0, HD : 3 * HD], wqkvT_r[0, :, 0, HD : 3 * HD]
            )
            pt_sb = persist.tile([P, HD], BF16)
            nc.gpsimd.dma_start(pt_sb[:], PT[:])
            nc.scalar.dma_start(
                w_cur[:, 0, 3 * HD : 4 * HD],
                wqkvT_r[0, :, 0, 3 * HD : 4 * HD],
            )
            # first wave: q/k columns only (the v columns aren't needed
            # until after the q/k matmul groups)
            for ko in range(1, KO):
                nc.sync.dma_start(xt0[:, ko], xT_r[:, ko, 0:SC])
                eng = nc.scalar if ko % 2 else nc.gpsimd
                eng.dma_start(
                    w_cur[:, ko, 0 : 4 * HD], wqkvT_r[0, :, ko, 0 : 4 * HD]
                )
            cs0 = load_cs(0, nc.scalar)
            ones_sb = persist.tile([P, P], BF16)
            nc.gpsimd.dma_start(ones_sb[:], ones[:])
            mask_sb = persist.tile([P, P], F32)
            nc.gpsimd.dma_start(mask_sb[:], maskT[:])
            # second wave: v columns (SP/Pool interleaved)
            for ko in range(KO):
                eng = nc.gpsimd if ko % 2 else nc.sync
                eng.dma_start(
                    w_cur[:, ko, 4 * HD : 6 * HD],
                    wqkvT_r[0, :, ko, 4 * HD : 6 * HD],
                )
            wo_cur = None

            pending = deque()
            drain_ct = [0]
            finish_prev = None

            def drain(n=1):
                k = 0
                while pending and k < n:
                    pending.popleft()()
                    drain_ct[0] += 1
                    k += 1

            for hb in range(NHB):
                w_sb = w_cur
                wo_sb = wo_cur

                # ---- per-pair persistent k/v ----
                kT_sb = kvq.tile([P, 2, S], BF16, tag="kT")
                v_sb = kvq.tile([P, NST, 2 * HD], BF16, tag="v")

                def project_chunk(sc, xt, cos_t, sin_t, finish_prev=None):
                    qT_dst = qtp.tile([P, 2, SC], BF16, tag="qTc")
                    ssl = slice(sc * SC, (sc + 1) * SC)

                    for hl in range(2):
                        # q into bank 0, k into bank 1 of one 2-bank tile
                        qk = s2p.tile([P, 2, SC], F32, tag="s2")
                        for t in range(2):
                            wcols = slice(
                                (2 * hl + t) * HD, (2 * hl + t + 1) * HD
                            )
                            for ko in range(KO):
                                nc.tensor.matmul(
                                    qk[:, t, :],
                                    w_sb[:, ko, wcols],
                                    xt[:, ko],
                                    start=(ko == 0),
                                    stop=(ko == KO - 1),
                                )
                        if hl == 0 and finish_prev is not None:
                            # previous chunk's softmax finish overlaps this
                            # dense matmul stretch (hides the denominator-
                            # accumulate tail + reciprocal latency)
                            finish_prev()
                        plain = scr.tile([P, 2, SC], BF16, tag="plain")
                        nc.scalar.copy(plain[:], qk[:])
                        if len(pending) > 14:
                            drain(1)
                        rot = s2p.tile([P, 2, SC], F32, tag="s2")
                        for t in range(2):
                            nc.tensor.matmul(
                                rot[:, t, :], pt_sb[:], plain[:, t, :]
                            )
                        # rope: dst = plain*cos + rot*sin, per t
                        for t in range(2):
                            pc = scr.tile([P, SC], F32, tag="pc")
                            nc.gpsimd.tensor_mul(
                                pc[:], plain[:, t, :], cos_t[:]
                            )
                            tmp2 = scr.tile([P, SC], F32, tag="tmp2")
                            nc.vector.tensor_mul(
                                tmp2[:], rot[:, t, :], sin_t[:]
                            )
                            dst = (
                                qT_dst[:, hl, :]
                                if t == 0
                                else kT_sb[:, hl, ssl]
                            )
                            nc.vector.tensor_add(dst, pc[:], tmp2[:])
                        if len(pending) > 14:
                            drain(1)

                    for sti in range(NSUB):
                        st = sc * NSUB + sti
                        lsl = slice(sti * P, (sti + 1) * P)
                        psv = s2p.tile([P, 2, SC], F32, tag="s2")
                        for ko in range(KO):
                            nc.tensor.matmul(
                                psv[:, 0, : 2 * HD],
                                xt[:, ko, lsl],
                                w_sb[:, ko, 4 * HD : 6 * HD],
                                start=(ko == 0),
                                stop=(ko == KO - 1),
                            )
                        nc.scalar.copy(v_sb[:, st, :], psv[:, 0, : 2 * HD])
                        if len(pending) > 14:
                            drain(1)
                    return qT_dst

                def attend_chunk(qc, qT_cur, outT_qc, hot=False):
                    """Attention for query chunk qc; both heads of the pair
                    share 2-bank score tiles so exp / denominator-accum /
                    reciprocal run once per k-block.  Fully-masked columns
                    of diagonal blocks are never computed.  One pending
                    wo-filler closure drains per k-block.  The softmax
                    denominator accumulates over two independent chains
                    (even k-blocks on DVE, odd on Pool) so neither engine's
                    latency serializes the chain; the broadcast/reciprocal/
                    normalize is returned as a closure the caller overlaps
                    with the next chunk's projection matmuls."""
                    nkb = (qc + 1) * NSUB
                    o_ps = {
                        hl: ops.tile([P, SC], F32, tag="o", name=f"o_ps{hl}")
                        for hl in range(2)
                    }
                    acc_a = accp.tile([P, 2, SC], BF16, tag="acca")
                    stile = {}

                    def tstart(kb):
                        j = kb - qc * NSUB
                        return j * P if j > 0 else 0

                    def emit_scores(kb):
                        t2 = s2p.tile([P, 2, SC], F32, tag="s2")
                        c0 = tstart(kb)
                        for hl in range(2):
                            nc.tensor.matmul(
                                t2[:, hl, c0:],
                                kT_sb[:, hl, kb * P : (kb + 1) * P],
                                qT_cur[:, hl, c0:],
                                skip_group_check=True,
                            )
                        j = kb - qc * NSUB
                        if j >= 0:
                            band = slice(j * P, (j + 1) * P)
                            for hl in range(2):
                                nc.vector.tensor_add(
                                    t2[:, hl, band], t2[:, hl, band],
                                    mask_sb[:],
                                )
                        stile[kb] = t2

                    emit_scores(0)
                    if nkb > 1:
                        emit_scores(1)
                    for kb in range(nkb):
                        c0 = tstart(kb)
                        e2 = expp.tile([P, 2, SC], BF16, tag="e2")
                        nc.scalar.activation(
                            e2[:, :, c0:], stile.pop(kb)[:, :, c0:], AF.Exp,
                            scale=inv_sqrt_hd,
                        )
                        for hl in range(2):
                            nc.tensor.matmul(
                                o_ps[hl][:, c0:],
                                v_sb[:, kb, hl * HD : (hl + 1) * HD],
                                e2[:, hl, c0:],
                                start=(kb == 0),
                                stop=(kb == nkb - 1),
                                skip_group_check=True,
                            )
                        # denominator accumulate: uniform-bf16 chain on DVE
                        # (mixed-width tensor ops run ~2.5x slower)
                        if kb == 0:
                            nc.vector.tensor_copy(acc_a[:], e2[:])
                        else:
                            nc.vector.tensor_add(
                                acc_a[:, :, c0:], acc_a[:, :, c0:],
                                e2[:, :, c0:],
                            )
                        if kb + 2 < nkb:
                            emit_scores(kb + 2)
                        drain(2 if len(pending) > 6 else 1)

                    drain(3)

                    def finish():
                        d2 = s2p.tile([P, 2, SC], F32, tag="s2")
                        for hl in range(2):
                            nc.tensor.matmul(
                                d2[:, hl, :], ones_sb[:], acc_a[:, hl, :]
                            )
                        recip2 = scr.tile([P, 2, SC], F32, tag="recip2")
                        nc.vector.reciprocal_approx_fast(recip2[:], d2[:])
                        for hl in range(2):
                            nc.vector.tensor_mul(
                                outT_qc[:, hl, :], o_ps[hl][:],
                                recip2[:, hl, :],
                            )

                    return finish

                def make_out_fillers(hb, qc, outT_qc, wo_sb):
                    """One closure per (s-tile, d-chunk) block of the wo
                    projection for query chunk qc: 2 accumulating matmuls,
                    a PSUM->SBUF copy (alternating ACT/DVE), and the
                    output DMA.  The very last s-tile's DMAs split 4-ways
                    so the final transfer doesn't trail the kernel."""
                    work = []
                    last = hb == NHB - 1 and qc == NQC - 1
                    for sti in range(NSUB):
                        st = qc * NSUB + sti
                        stsl = slice(sti * P, (sti + 1) * P)
                        split = 2 if (last and sti >= NSUB - 2) else 1
                        for dc in range(D // SC):
                            dsl = slice(dc * SC, (dc + 1) * SC)

                            def blk(hb=hb, st=st, stsl=stsl, dsl=dsl,
                                    outT_qc=outT_qc, wo_sb=wo_sb,
                                    split=split, last=last):
                                # the final drain has no attention work to
                                # hide the PSUM->SBUF copy latency, so
                                # alternate its y tiles between the yps
                                # pool and the (now idle) score pool for
                                # an effectively 4-deep rotation
                                if last and drain_ct[0] % 2 == 1:
                                    t2 = s2p.tile([P, 2, SC], F32, tag="s2")
                                    y_ps = t2[:, 0, :]
                                else:
                                    y_ps = yps.tile([P, SC], F32, tag="y")
                                for hl in range(2):
                                    nc.tensor.matmul(
                                        y_ps[:],
                                        outT_qc[:, hl, stsl],
                                        wo_sb[:, hl, dsl],
                                        start=(hl == 0),
                                        stop=(hl == 1),
                                    )
                                y_sb = yop.tile([P, SC], BF16, tag="ysb")
                                if drain_ct[0] % 2 == 0:
                                    nc.scalar.copy(y_sb[:], y_ps[:])
                                else:
                                    nc.vector.tensor_copy(y_sb[:], y_ps[:])
                                # final drain: no compute follows, so the
                                # ACT queue can absorb half the output-DMA
                                # issues (SP alone serializes ~650ns each)
                                deng = (
                                    nc.scalar
                                    if (last and drain_ct[0] % 2 == 1)
                                    else nc.sync
                                )
                                w_ = SC // split
                                for s_ in range(split):
                                    deng.dma_start(
                                        y[hb, st * P : (st + 1) * P,
                                          dsl.start + s_ * w_ :
                                          dsl.start + (s_ + 1) * w_],
                                        y_sb[:, s_ * w_ : (s_ + 1) * w_],
                                    )

                            work.append(blk)
                    return work

                xt_cur, cs_cur = (xt0, cs0) if hb == 0 else (xt_nx, cs_nx)
                for sc in range(NQC):
                    if hb == 0 and sc == 2:
                        w_cur = load_w(1)
                        wo_cur = load_wo(1, nc.sync)

                    # prefetch next x chunk first so its transfers aren't
                    # queued behind this chunk's y-output DMAs on SP
                    if sc < NQC - 1:
                        xt_nx = load_x(sc + 1)
                        cs_nx = load_cs(sc + 1)
                    elif hb == 0:
                        xt_nx = load_x(0)
                        cs_nx = load_cs(0)
                    if hb == 0 and sc == 0:
                        wo_cur = load_wo(0, nc.sync)
                        wo_sb = wo_cur
                    qT_cur = project_chunk(sc, xt_cur, *cs_cur, finish_prev)
                    finish_prev = None
                    outT_qc = outqp.tile([P, 2, SC], BF16, tag="outq")
                    finish_prev = attend_chunk(
                        sc, qT_cur, outT_qc,
                        hot=(hb == NHB - 1 and sc == NQC - 1),
                    )
                    pending.extend(make_out_fillers(hb, sc, outT_qc, wo_sb))
                    if sc < NQC - 1 or hb == 0:
                        xt_cur, cs_cur = xt_nx, cs_nx

            finish_prev()
            while pending:
                pending.popleft()()

    nc.compile()
    return nc


_NC_CACHE = {}


def _get_nc():
    if "nc" not in _NC_CACHE:
        _NC_CACHE["nc"] = _build_core_kernel()
    return _NC_CACHE["nc"]


def _rope_perm_T() -> np.ndarray:
    # rotate_half as a matrix: (P_rh @ q)[d] = -q[d+HD/2] for d < HD/2,
    # q[d-HD/2] otherwise.  Returns P_rh.T for use as matmul lhsT.
    P_rh = np.zeros((HD, HD), dtype=np.float32)
    half = HD // 2
    for i in range(half):
        P_rh[i, half + i] = -1.0
        P_rh[half + i, i] = 1.0
    return np.ascontiguousarray(P_rh.T)


def _is_causal(m: np.ndarray) -> bool:
    tril = np.tril(np.ones((S, S), dtype=bool))
    if not np.all(m[tril] == 0.0):
        return False
    upper = m[~tril]
    return bool(upper.size == 0 or np.all(upper <= -1.0e8))


# module-level: results of the last traced run (for test harnesses)
last_exec_time_ns = None
last_profile_json = None


def kernel(x, cos, sin, mask, wq, wk, wv, wo, _trace=False):
    x = np.asarray(x, dtype=np.float32)
    cos = np.asarray(cos, dtype=np.float32)
    sin = np.asarray(sin, dtype=np.float32)
    mask = np.asarray(mask, dtype=np.float32)
    wq = np.asarray(wq, dtype=np.float32)
    wk = np.asarray(wk, dtype=np.float32)
    wv = np.asarray(wv, dtype=np.float32)
    wo = np.asarray(wo, dtype=np.float32)

    m2d = mask.reshape(S, S)
    assert _is_causal(m2d), "this kernel only supports the causal mask"
    nc = _get_nc()

    scale = np.float32(np.sqrt(HD))
    # triangular [k, q] mask for the 128x128 diagonal blocks
    mask_tri = np.ascontiguousarray((m2d[:P, :P] * scale).T)
    cosT = np.ascontiguousarray(cos.T, dtype=np.float32)
    sinT = np.ascontiguousarray(sin.T, dtype=np.float32)
    ptT = _rope_perm_T().astype(ml_dtypes.bfloat16)
    ones = np.ones((P, P), dtype=np.float32).astype(ml_dtypes.bfloat16)

    bf = ml_dtypes.bfloat16
    xT = [np.ascontiguousarray(x[b].T).astype(bf) for b in range(B)]

    in_maps = []
    for c in range(N_CORES):
        b = c // (N_CORES // B)
        hg = c % (N_CORES // B)
        rows = slice(hg * HW, (hg + 1) * HW)
        # pack per head-pair: [q_h0 | k_h0 | q_h1 | k_h1 | v_h0 | v_h1]
        packs = []
        for hbp in range(H_LOC // 2):
            cols = []
            for hl in range(2):
                h = hg * H_LOC + hbp * 2 + hl
                cols.append(wq[h * HD : (h + 1) * HD].T)
                cols.append(wk[h * HD : (h + 1) * HD].T)
            for hl in range(2):
                h = hg * H_LOC + hbp * 2 + hl
                cols.append(wv[h * HD : (h + 1) * HD].T)
            packs.append(np.concatenate(cols, axis=1))
        wqkvT = np.stack(packs).astype(bf)
        in_maps.append(
            {
                "xT": xT[b],
                "wqkvT": wqkvT,
                "woT": np.ascontiguousarray(wo[:, rows].T).astype(bf),
                "cosT": cosT,
                "sinT": sinT,
                "PT": ptT,
                "ones": ones,
                "maskT": mask_tri.astype(np.float32),
            }
        )

    kw = {}
    if _trace:
        kw = dict(trace=True)
    res = run_bass_kernel_spmd(
        nc, in_maps, core_ids=list(range(N_CORES)), **kw
    )
    global last_exec_time_ns, last_profile_json
    last_exec_time_ns = res.exec_time_ns
    last_profile_json = res.profile_json

    out = np.empty((B, S, D), dtype=np.float32)
    gs = N_CORES // B
    for b in range(B):
        acc = None
        for g in range(gs):
            yc = res.results[b * gs + g]["y"].astype(np.float32)
            part = yc[0] + yc[1]
            acc = part if acc is None else acc + part
        out[b] = acc
    return out


# revision 55
# speedup vs baseline: 1.0124x; 1.0124x over previous
"""Trainium2 8-core kernel for nn_Attention_27530740367526.

Multi-head causal attention (B=2, S=2048, D=2048, H=16, HD=128, fp32) with
RoPE, sharded batch x head-group across 8 NeuronCores: core c handles batch
c//4 and heads [4*(c%4), 4*(c%4)+4).  Each core computes q/k/v projections
(+RoPE), attention for its heads, and the slice of the wo projection those
heads feed — per-head-pair partial [S, D] outputs.  The host sums the 8
partials per batch (row-parallel wo "all-reduce" as a host-side unshard).

On-device everything lives in "transposed land": qT/kT are [head_dim, seq]
with head-dim on partitions, so scores come out transposed ([k, q]), and
PV / wo consume natural layouts with zero on-device transposes.  RoPE's
rotate-half is a 128x128 permutation matmul on the PE.

Schedule notes: all matmul operands and the y partials are bfloat16 (half
the DMA bytes, fast weight loads; attention math accumulates in fp32
PSUM).  The two heads of a pair share two-bank PSUM tiles ([P, 2, 512])
so exp / denominator-accumulate / reciprocal run once per k-block instead
of twice (halves the fixed per-instruction overhead on ACT/DVE).  The
softmax denominator is accumulated across k-blocks as a uniform-bf16
chain on DVE (mixed-width DVE tensor ops run ~2.5x slower) and broadcast
with a single ones-matmul per chunk whose reciprocal/normalize is
deferred into the next chunk's projection matmuls.  Fully-masked columns
of diagonal score blocks are never computed.  The next head-pair's
weights prefetch during the current pair's attention (double-buffered) so
the PE never starves at the pair boundary (keeps the HAM clock-gate at
8/8).  wo-projection filler blocks drain 1-2 per attention k-block where
the PE would otherwise wait on exp; leftovers carry across the pair
boundary.  Startup DMAs issue fine-grained and need-ordered across the
SP/ACT/Pool DGE queues (the ACT queue carries no late DMAs — a DMA issue
can block many us on ring credits, stalling compute queued behind it).
"""

import sys

if "/opt/trn_rl_repo" not in sys.path:
    sys.path.insert(0, "/opt/trn_rl_repo")

from collections import deque

import numpy as np
import ml_dtypes

import concourse.bacc as bacc
import concourse.mybir as mybir
import concourse.tile as tile
from concourse.bass_utils import run_bass_kernel_spmd

F32 = mybir.dt.float32
F32R = mybir.dt.float32r
BF16 = mybir.dt.bfloat16
AF = mybir.ActivationFunctionType

N_HEADS = 16
N_CORES = 8
B, S, D = 2, 2048, 2048
HD = D // N_HEADS
H_LOC = N_HEADS // (N_CORES // B)  # 4 heads per core
HW = H_LOC * HD                    # 512 q/k/v columns per core
SC = 512                           # seq chunk (matmul moving free dim)
P = 128


def _round_f32r(x: np.ndarray) -> np.ndarray:
    """Host-side fp32 -> float32r rounding (RNE to 11 explicit mantissa
    bits); bit-exact with the device DVE rounding."""
    xi = np.ascontiguousarray(x, dtype=np.float32).view(np.uint32)
    nbits = 12
    lo = np.uint32((1 << nbits) - 1)
    half = np.uint32(1 << (nbits - 1))
    rem = xi & lo
    up = (rem > half) | ((rem == half) & (((xi >> nbits) & 1) == 1))
    r = (xi & ~lo) + np.where(up, np.uint32(1 << nbits), np.uint32(0))
    return r.view(np.float32)


def _build_core_kernel():
    KO = D // P            # 16 contraction subtiles for projections
    NQC = S // SC          # 4 q-chunks
    NSUB = SC // P         # 4 128-blocks per chunk
    NST = S // P           # 16 s-tiles
    NHB = H_LOC // 2       # head pairs
    inv_sqrt_hd = 1.0 / float(np.sqrt(HD))

    nc = bacc.Bacc(None, target_bir_lowering=False)

    xT = nc.dram_tensor("xT", [D, S], BF16, kind="ExternalInput")
    wqkvT = nc.dram_tensor(
        "wqkvT", [NHB, D, 6 * HD], BF16, kind="ExternalInput"
    )
    woT = nc.dram_tensor("woT", [HW, D], BF16, kind="ExternalInput")
    cosT = nc.dram_tensor("cosT", [HD, S], F32, kind="ExternalInput")
    sinT = nc.dram_tensor("sinT", [HD, S], F32, kind="ExternalInput")
    PT = nc.dram_tensor("PT", [HD, HD], BF16, kind="ExternalInput")
    ones = nc.dram_tensor("ones", [P, P], BF16, kind="ExternalInput")
    # triangular additive mask for the 128x128 diagonal score blocks,
    # [k, q] orientation, pre-scaled by sqrt(HD)
    maskT = nc.dram_tensor("maskT", [P, P], F32, kind="ExternalInput")
    y = nc.dram_tensor("y", [NHB, S, D], BF16, kind="ExternalOutput")

    xT_r = xT.rearrange("(ko ki) s -> ki ko s", ki=P)
    wqkvT_r = wqkvT.rearrange("hb (ko ki) c -> hb ki ko c", ki=P)
    woT_r = woT.rearrange("(h ki) d -> ki h d", ki=P)

    with tile.TileContext(nc) as tc:
        with (
            tc.tile_pool(name="persist", bufs=1) as persist,
            tc.tile_pool(name="wpool", bufs=2) as wpool,
            tc.tile_pool(name="kvq", bufs=1) as kvq,
            tc.tile_pool(name="xa", bufs=1) as xa,
            tc.tile_pool(name="cs", bufs=2) as cspool,
            tc.tile_pool(name="scr", bufs=2) as scr,
            tc.tile_pool(name="exps", bufs=5) as expp,
            tc.tile_pool(name="qtp", bufs=2) as qtp,
            tc.tile_pool(name="outq", bufs=3) as outqp,
            tc.tile_pool(name="accs", bufs=2) as accp,
            tc.tile_pool(name="yo", bufs=6) as yop,
            tc.tile_pool(name="ps2", bufs=2, space="PSUM") as s2p,
            tc.tile_pool(name="ops", bufs=2, space="PSUM") as ops,
            tc.tile_pool(name="yps", bufs=2, space="PSUM") as yps,
        ):
            def load_cs(sc, eng=None):
                # SP queue by default: a DMA issue on the ACT queue can
                # block for many us on ring credits, stalling the compute
                # ops (plain copies / exps) queued behind it.  Two pieces
                # per tensor so a single transfer isn't 11us long.
                eng = eng or nc.sync
                cos_t = cspool.tile([P, SC], F32, tag="cos")
                sin_t = cspool.tile([P, SC], F32, tag="sin")
                for h in range(2):
                    l = slice(h * 256, (h + 1) * 256)
                    g = slice(sc * SC + h * 256, sc * SC + (h + 1) * 256)
                    eng.dma_start(cos_t[:, l], cosT[:, g])
                    eng.dma_start(sin_t[:, l], sinT[:, g])
                return cos_t, sin_t

            def load_x(sc):
                """x chunk DMAs into the persistent full-x tile (SP
                queue); x stays resident so pair 2 never re-reads it."""
                ssl = slice(sc * SC, (sc + 1) * SC)
                for ko in range(KO):
                    nc.sync.dma_start(xfull[:, ko, ssl], xT_r[:, ko, ssl])

            def load_w(hb):
                """wqkv DMAs for head pair hb (SP queue)."""
                w_sb = wpool.tile([P, KO, 6 * HD], BF16, tag="w")
                for ko in range(KO):
                    nc.sync.dma_start(w_sb[:, ko, :], wqkvT_r[hb, :, ko, :])
                return w_sb

            def load_wo(hb, eng):
                wo_sb = wpool.tile([P, 2, D], BF16, tag="wo")
                for hl in range(2):
                    for dc in range(4):
                        dsl = slice(dc * SC, (dc + 1) * SC)
                        eng.dma_start(
                            wo_sb[:, hl, dsl],
                            woT_r[:, hb * 2 + hl, dsl],
                        )
                return wo_sb

            # ---- startup: spread w/x issue across the SP / ACT / Pool
            # DGE queues, interleaved per-ko, so the first matmul group
            # can begin within a few us and streams at aggregate DMA bw.
            # Issue order is by need-time: x0 + w q/k wave first, then the
            # v columns, then x1 / cos/sin / wo.  The ACT queue carries
            # only the first w wave (its later compute must not sit
            # behind ring-credit-blocked DMA issues).
            w_cur = wpool.tile([P, KO, 6 * HD], BF16, tag="w", name="w_sb")
            xfull = xa.tile([P, KO, S], BF16, tag="xt", name="xfull")
            # the first q-weight piece + x pieces lead their queues so the
            # first matmul group can start as early as possible
            nc.sync.dma_start(w_cur[:, 0, 0:HD], wqkvT_r[0, :, 0, 0:HD])
            for i in range(2):
                nc.sync.dma_start(
                    xfull[:, 0, i * P : (i + 1) * P],
                    xT_r[:, 0, i * P : (i + 1) * P],
                )
            for i in range(2, 4):
                nc.gpsimd.dma_start(
                    xfull[:, 0, i * P : (i + 1) * P],
                    xT_r[:, 0, i * P : (i + 1) * P],
                )
            nc.scalar.dma_start(
                w_cur[:, 0, HD : 3 * HD], wqkvT_r[0, :, 0, HD : 3 * HD]
            )
            pt_sb = persist.tile([P, HD], BF16)
            nc.gpsimd.dma_start(pt_sb[:], PT[:])
            nc.scalar.dma_start(
                w_cur[:, 0, 3 * HD : 4 * HD],
                wqkvT_r[0, :, 0, 3 * HD : 4 * HD],
            )
            # first wave: q/k columns only (the v columns aren't needed
            # until after the q/k matmul groups)
            for ko in range(1, KO):
                nc.sync.dma_start(xfull[:, ko, 0:SC], xT_r[:, ko, 0:SC])
                eng = nc.scalar if ko % 2 else nc.gpsimd
                eng.dma_start(
                    w_cur[:, ko, 0 : 4 * HD], wqkvT_r[0, :, ko, 0 : 4 * HD]
                )
            cs0 = load_cs(0, nc.scalar)
            ones_sb = persist.tile([P, P], BF16)
            nc.gpsimd.dma_start(ones_sb[:], ones[:])
            mask_sb = persist.tile([P, P], F32)
            nc.gpsimd.dma_start(mask_sb[:], maskT[:])
            # second wave: v columns (SP/Pool interleaved)
            for ko in range(KO):
                eng = nc.gpsimd if ko % 2 else nc.sync
                eng.dma_start(
                    w_cur[:, ko, 4 * HD : 6 * HD],
                    wqkvT_r[0, :, ko, 4 * HD : 6 * HD],
                )
            wo_cur = None

            pending = deque()
            drain_ct = [0]
            finish_prev = None

            def drain(n=1):
                k = 0
                while pending and k < n:
                    pending.popleft()()
                    drain_ct[0] += 1
                    k += 1

            for hb in range(NHB):
                w_sb = w_cur
                wo_sb = wo_cur

                # ---- per-pair persistent k/v ----
                kT_sb = kvq.tile([P, 2, S], BF16, tag="kT")
                v_sb = kvq.tile([P, NST, 2 * HD], BF16, tag="v")

                def project_chunk(sc, cos_t, sin_t, finish_prev=None):
                    qT_dst = qtp.tile([P, 2, SC], BF16, tag="qTc")
                    ssl = slice(sc * SC, (sc + 1) * SC)

                    for hl in range(2):
                        # q into bank 0, k into bank 1 of one 2-bank tile
                        qk = s2p.tile([P, 2, SC], F32, tag="s2")
                        for t in range(2):
                            wcols = slice(
                                (2 * hl + t) * HD, (2 * hl + t + 1) * HD
                            )
                            for ko in range(KO):
                                nc.tensor.matmul(
                                    qk[:, t, :],
                                    w_sb[:, ko, wcols],
                                    xfull[:, ko, ssl],
                                    start=(ko == 0),
                                    stop=(ko == KO - 1),
                                )
                        if hl == 0 and finish_prev is not None:
                            # previous chunk's softmax finish overlaps this
                            # dense matmul stretch (hides the denominator-
                            # accumulate tail + reciprocal latency)
                            finish_prev()
                        plain = scr.tile([P, 2, SC], BF16, tag="plain")
                        nc.scalar.copy(plain[:], qk[:])
                        if len(pending) > 14:
                            drain(1)
                        rot = s2p.tile([P, 2, SC], F32, tag="s2")
                        for t in range(2):
                            nc.tensor.matmul(
                                rot[:, t, :], pt_sb[:], plain[:, t, :]
                            )
                        # rope: dst = plain*cos + rot*sin, per t
                        for t in range(2):
                            pc = scr.tile([P, SC], F32, tag="pc")
                            nc.gpsimd.tensor_mul(
                                pc[:], plain[:, t, :], cos_t[:]
                            )
                            tmp2 = scr.tile([P, SC], F32, tag="tmp2")
                            nc.vector.tensor_mul(
                                tmp2[:], rot[:, t, :], sin_t[:]
                            )
                            dst = (
                                qT_dst[:, hl, :]
                                if t == 0
                                else kT_sb[:, hl, ssl]
                            )
                            nc.vector.tensor_add(dst, pc[:], tmp2[:])
                        if len(pending) > 14:
                            drain(1)

                    for sti in range(NSUB):
                        st = sc * NSUB + sti
                        lsl = slice(sti * P, (sti + 1) * P)
                        psv = s2p.tile([P, 2, SC], F32, tag="s2")
                        for ko in range(KO):
                            nc.tensor.matmul(
                                psv[:, 0, : 2 * HD],
                                xfull[:, ko, sc * SC + sti * P :
                                       sc * SC + (sti + 1) * P],
                                w_sb[:, ko, 4 * HD : 6 * HD],
                                start=(ko == 0),
                                stop=(ko == KO - 1),
                            )
                        nc.scalar.copy(v_sb[:, st, :], psv[:, 0, : 2 * HD])
                        if len(pending) > 14:
                            drain(1)
                    return qT_dst

                def attend_chunk(qc, qT_cur, outT_qc, hot=False):
                    """Attention for query chunk qc; both heads of the pair
                    share 2-bank score tiles so exp / denominator-accum /
                    reciprocal run once per k-block.  Fully-masked columns
                    of diagonal blocks are never computed.  One pending
                    wo-filler closure drains per k-block.  The softmax
                    denominator accumulates over two independent chains
                    (even k-blocks on DVE, odd on Pool) so neither engine's
                    latency serializes the chain; the broadcast/reciprocal/
                    normalize is returned as a closure the caller overlaps
                    with the next chunk's projection matmuls."""
                    nkb = (qc + 1) * NSUB
                    o_ps = {
                        hl: ops.tile([P, SC], F32, tag="o", name=f"o_ps{hl}")
                        for hl in range(2)
                    }
                    acc_a = accp.tile([P, 2, SC], BF16, tag="acca")
                    stile = {}

                    def tstart(kb):
                        j = kb - qc * NSUB
                        return j * P if j > 0 else 0

                    def emit_scores(kb):
                        t2 = s2p.tile([P, 2, SC], F32, tag="s2")
                        c0 = tstart(kb)
                        for hl in range(2):
                            nc.tensor.matmul(
                                t2[:, hl, c0:],
                                kT_sb[:, hl, kb * P : (kb + 1) * P],
                                qT_cur[:, hl, c0:],
                                skip_group_check=True,
                            )
                        j = kb - qc * NSUB
                        if j >= 0:
                            band = slice(j * P, (j + 1) * P)
                            for hl in range(2):
                                nc.vector.tensor_add(
                                    t2[:, hl, band], t2[:, hl, band],
                                    mask_sb[:],
                                )
                        stile[kb] = t2

                    emit_scores(0)
                    if nkb > 1:
                        emit_scores(1)
                    for kb in range(nkb):
                        c0 = tstart(kb)
                        e2 = expp.tile([P, 2, SC], BF16, tag="e2")
                        nc.scalar.activation(
                            e2[:, :, c0:], stile.pop(kb)[:, :, c0:], AF.Exp,
                            scale=inv_sqrt_hd,
                        )
                        for hl in range(2):
                            nc.tensor.matmul(
                                o_ps[hl][:, c0:],
                                v_sb[:, kb, hl * HD : (hl + 1) * HD],
                                e2[:, hl, c0:],
                                start=(kb == 0),
                                stop=(kb == nkb - 1),
                                skip_group_check=True,
                            )
                        # denominator accumulate: uniform-bf16 chain on DVE
                        # (mixed-width tensor ops run ~2.5x slower)
                        if kb == 0:
                            nc.vector.tensor_copy(acc_a[:], e2[:])
                        else:
                            nc.vector.tensor_add(
                                acc_a[:, :, c0:], acc_a[:, :, c0:],
                                e2[:, :, c0:],
                            )
                        if kb + 2 < nkb:
                            emit_scores(kb + 2)
                        drain(2 if len(pending) > 6 else 1)

                    drain(2)

                    def finish():
                        d2 = s2p.tile([P, 2, SC], F32, tag="s2")
                        for hl in range(2):
                            nc.tensor.matmul(
                                d2[:, hl, :], ones_sb[:], acc_a[:, hl, :]
                            )
                        recip2 = scr.tile([P, 2, SC], F32, tag="recip2")
                        nc.vector.reciprocal_approx_fast(recip2[:], d2[:])
                        for hl in range(2):
                            nc.vector.tensor_mul(
                                outT_qc[:, hl, :], o_ps[hl][:],
                                recip2[:, hl, :],
                            )

                    return finish

                def make_out_fillers(hb, qc, outT_qc, wo_sb):
                    """One closure per (s-tile, d-chunk) block of the wo
                    projection for query chunk qc: 2 accumulating matmuls,
                    a PSUM->SBUF copy (alternating ACT/DVE), and the
                    output DMA.  The very last s-tile's DMAs split 4-ways
                    so the final transfer doesn't trail the kernel."""
                    work = []
                    last = hb == NHB - 1 and qc == NQC - 1
                    for sti in range(NSUB):
                        st = qc * NSUB + sti
                        stsl = slice(sti * P, (sti + 1) * P)
                        split = 2 if (last and sti >= NSUB - 2) else 1
                        for dc in range(D // SC):
                            dsl = slice(dc * SC, (dc + 1) * SC)

                            def blk(hb=hb, st=st, stsl=stsl, dsl=dsl,
                                    outT_qc=outT_qc, wo_sb=wo_sb,
                                    split=split, last=last):
                                # the final drain has no attention work to
                                # hide the PSUM->SBUF copy latency, so
                                # alternate its y tiles between the yps
                                # pool and the (now idle) score pool for
                                # an effectively 4-deep rotation
                                if last and drain_ct[0] % 2 == 1:
                                    t2 = s2p.tile([P, 2, SC], F32, tag="s2")
                                    y_ps = t2[:, 0, :]
                                else:
                                    y_ps = yps.tile([P, SC], F32, tag="y")
                                for hl in range(2):
                                    nc.tensor.matmul(
                                        y_ps[:],
                                        outT_qc[:, hl, stsl],
                                        wo_sb[:, hl, dsl],
                                        start=(hl == 0),
                                        stop=(hl == 1),
                                    )
                                y_sb = yop.tile([P, SC], BF16, tag="ysb")
                                if drain_ct[0] % 2 == 0:
                                    nc.scalar.copy(y_sb[:], y_ps[:])
                                else:
                                    nc.vector.tensor_copy(y_sb[:], y_ps[:])
                                # final drain: no compute follows, so the
                                # ACT queue can absorb half the output-DMA
                                # issues (SP alone serializes ~650ns each)
                                deng = (
                                    nc.scalar
                                    if (last and drain_ct[0] % 2 == 1)
                                    else nc.sync
                                )
                                w_ = SC // split
                                for s_ in range(split):
                                    deng.dma_start(
                                        y[hb, st * P : (st + 1) * P,
                                          dsl.start + s_ * w_ :
                                          dsl.start + (s_ + 1) * w_],
                                        y_sb[:, s_ * w_ : (s_ + 1) * w_],
                                    )

                            work.append(blk)
                    return work

                cs_cur = cs0 if hb == 0 else cs_nx
                for sc in range(NQC):
                    if hb == 0 and sc == 2:
                        w_cur = load_w(1)
                        wo_cur = load_wo(1, nc.sync)

                    # prefetch next x chunk first so its transfers aren't
                    # queued behind this chunk's y-output DMAs on SP
                    if sc < NQC - 1:
                        if hb == 0:
                            load_x(sc + 1)
                        cs_nx = load_cs(sc + 1)
                    elif hb == 0:
                        cs_nx = load_cs(0)
                    if hb == 0 and sc == 0:
                        wo_cur = load_wo(0, nc.sync)
                        wo_sb = wo_cur
                    qT_cur = project_chunk(sc, *cs_cur, finish_prev)
                    finish_prev = None
                    outT_qc = outqp.tile([P, 2, SC], BF16, tag="outq")
                    finish_prev = attend_chunk(
                        sc, qT_cur, outT_qc,
                        hot=(hb == NHB - 1 and sc == NQC - 1),
                    )
                    pending.extend(make_out_fillers(hb, sc, outT_qc, wo_sb))
                    if sc < NQC - 1 or hb == 0:
                        cs_cur = cs_nx

            finish_prev()
            while pending:
                pending.popleft()()

    nc.compile()
    return nc


_NC_CACHE = {}


def _get_nc():
    if "nc" not in _NC_CACHE:
        _NC_CACHE["nc"] = _build_core_kernel()
    return _NC_CACHE["nc"]


def _rope_perm_T() -> np.ndarray:
    # rotate_half as a matrix: (P_rh @ q)[d] = -q[d+HD/2] for d < HD/2,
    # q[d-HD/2] otherwise.  Returns P_rh.T for use as matmul lhsT.
    P_rh = np.zeros((HD, HD), dtype=np.float32)
    half = HD // 2
    for i in range(half):
        P_rh[i, half + i] = -1.0
        P_rh[half + i, i] = 1.0
    return np.ascontiguousarray(P_rh.T)


def _is_causal(m: np.ndarray) -> bool:
    tril = np.tril(np.ones((S, S), dtype=bool))
    if not np.all(m[tril] == 0.0):
        return False
    upper = m[~tril]
    return bool(upper.size == 0 or np.all(upper <= -1.0e8))


# module-level: results of the last traced run (for test harnesses)
last_exec_time_ns = None
last_profile_json = None


def kernel(x, cos, sin, mask, wq, wk, wv, wo, _trace=False):
    x = np.asarray(x, dtype=np.float32)
    cos = np.asarray(cos, dtype=np.float32)
    sin = np.asarray(sin, dtype=np.float32)
    mask = np.asarray(mask, dtype=np.float32)
    wq = np.asarray(wq, dtype=np.float32)
    wk = np.asarray(wk, dtype=np.float32)
    wv = np.asarray(wv, dtype=np.float32)
    wo = np.asarray(wo, dtype=np.float32)

    m2d = mask.reshape(S, S)
    assert _is_causal(m2d), "this kernel only supports the causal mask"
    nc = _get_nc()

    scale = np.float32(np.sqrt(HD))
    # triangular [k, q] mask for the 128x128 diagonal blocks
    mask_tri = np.ascontiguousarray((m2d[:P, :P] * scale).T)
    cosT = np.ascontiguousarray(cos.T, dtype=np.float32)
    sinT = np.ascontiguousarray(sin.T, dtype=np.float32)
    ptT = _rope_perm_T().astype(ml_dtypes.bfloat16)
    ones = np.ones((P, P), dtype=np.float32).astype(ml_dtypes.bfloat16)

    bf = ml_dtypes.bfloat16
    xT = [np.ascontiguousarray(x[b].T).astype(bf) for b in range(B)]

    in_maps = []
    for c in range(N_CORES):
        b = c // (N_CORES // B)
        hg = c % (N_CORES // B)
        rows = slice(hg * HW, (hg + 1) * HW)
        # pack per head-pair: [q_h0 | k_h0 | q_h1 | k_h1 | v_h0 | v_h1]
        packs = []
        for hbp in range(H_LOC // 2):
            cols = []
            for hl in range(2):
                h = hg * H_LOC + hbp * 2 + hl
                cols.append(wq[h * HD : (h + 1) * HD].T)
                cols.append(wk[h * HD : (h + 1) * HD].T)
            for hl in range(2):
                h = hg * H_LOC + hbp * 2 + hl
                cols.append(wv[h * HD : (h + 1) * HD].T)
            packs.append(np.concatenate(cols, axis=1))
        wqkvT = np.stack(packs).astype(bf)
        in_maps.append(
            {
                "xT": xT[b],
                "wqkvT": wqkvT,
                "woT": np.ascontiguousarray(wo[:, rows].T).astype(bf),
                "cosT": cosT,
                "sinT": sinT,
                "PT": ptT,
                "ones": ones,
                "maskT": mask_tri.astype(np.float32),
            }
        )

    kw = {}
    if _trace:
        kw = dict(trace=True)
    res = run_bass_kernel_spmd(
        nc, in_maps, core_ids=list(range(N_CORES)), **kw
    )
    global last_exec_time_ns, last_profile_json
    last_exec_time_ns = res.exec_time_ns
    last_profile_json = res.profile_json

    out = np.empty((B, S, D), dtype=np.float32)
    gs = N_CORES // B
    for b in range(B):
        acc = None
        for g in range(gs):
            yc = res.results[b * gs + g]["y"].astype(np.float32)
            part = yc[0] + yc[1]
            acc = part if acc is None else acc + part
        out[b] = acc
    return out
